# revision 10
# baseline (speedup 1.0000x reference)
"""Trainium2 Bass kernel for nn_BitModel (MLGRU step + BitGLU, ternary weights).

Strategy: pure data-parallel over the 4096 batch dim (512 rows per core,
zero collectives). Weights are ternarized exactly in f32 on the host
(sign(w) * (|w| >= 0.33)); the ternary values {-1, 0, +1} are exactly
representable in fp16, so device matmuls run at full 1-cycle/row TensorE
rate with zero weight quantization error.

Precision scheme (validated against the reference in numpy emulation,
rel err ~2e-4): activations that feed matmuls are split into fp16 hi+lo
pairs (x, gh, o), which doubles those contraction loops but keeps ~22-bit
effective mantissa; gu is a single fp16 stream pre-scaled by 1/16 (range),
un-scaled in the output epilogue. All accumulation is f32 in PSUM.

On-device dataflow is feature-major: every tensor lives in SBUF as
[128 partitions = feature % 128, feature_chunk, batch=512]. Each BitLinear is
out[j, b] = sum_k Wt[k, j] * act[k, b]; lhsT = pre-transposed weight tile
[k_part, j] (stationary), rhs = activation tile [k_part, batch] (moving,
N=512 = one PSUM bank). No transposes anywhere on device.
"""

import sys

sys.path.insert(0, "/opt/trn_rl_repo")

import numpy as np

import concourse.bass as bass
import concourse.mybir as mybir
import concourse.tile as tile
from concourse.vector_clock import ScopedClock

DIM = 2048
HID = 8192
BATCH = 4096
NCORES = 8
B = BATCH // NCORES  # 512 batch rows per core
P = 128
JC_D = DIM // P  # 16 feature chunks for DIM
JC_H = HID // P  # 64 feature chunks for HID
THRESH = 0.33
GU_SCALE = 16.0  # gu is stored as gu/16 in fp16 to stay inside fp16 range

F16 = mybir.dt.float16
F32 = mybir.dt.float32

# bias column layout in the packed [128, 208] bias tensor
COL_NF = 0  # -f_gate_b (negated: we compute 1-f = sigmoid(-(t+b)))
COL_C = 16
COL_G = 32
COL_O = 48
COL_U = 64
COL_G2 = 128
COL_Y = 192
N_BIAS_COLS = 208


def _patch_tile_drain():
    """This walrus build rejects instructions carrying >~2 attached sem
    waits ("Too many sync wait commands") and Tile's kernel-tail drain
    carries one wait per active logical proc. Re-emit those waits as
    standalone wait_ge instructions (1 wait each) before a wait-free
    drain."""
    if getattr(tile.TileContext, "_drain_patched", False):
        return

    def _drain_and_barrier(self, tick_clock, wait_clock):
        nc = self.nc
        probe = nc.sync.nop(nofuse=True)
        wait_clock.add_sem_waits(
            probe.ins, ScopedClock({None: tick_clock.global_clock})
        )
        si = probe.ins.sync_info
        waits = list(si.on_wait) if si else []
        if si:
            si.on_wait = []
        handles = {h.name: h for h in self.sems.allocated().values()}
        for w in waits:
            nc.sync.wait_ge(handles[w.ant_name], w.wait_value)
        nc.sync.drain()
        nc.all_engine_barrier()
        assert self.sems is not None
        popped = nc._tile_sem_poison_stack.pop()
        assert popped is self._sem_poison
        nc.clear_and_free_semaphores(list(self.sems.allocated().values()))
        nc.all_engine_barrier()

    tile.TileContext._drain_and_barrier = _drain_and_barrier
    tile.TileContext._drain_patched = True


_patch_tile_drain()


def _split_excess_waits(nc, cap=1):
    """This walrus build rejects instructions carrying more than ~2 attached
    sem waits ("Too many sync wait commands"). Tile attaches one wait per
    depended-on logical proc. Rewrite every instruction with >cap waits into
    a chain of single-wait InstEventSemaphore ops (what raw wait_ge emits,
    known-good) followed by the instruction keeping only `cap` waits."""
    ctr = 0
    for f in nc.m.functions:
        for bb in f.blocks:
            il = bb.instructions
            i = 0
            while i < len(il):
                inst = il[i]
                si = inst.sync_info
                waits = list(si.on_wait) if si else []
                if len(waits) > cap:
                    extra, keep = waits[:-cap], waits[-cap:]
                    evs = []
                    for w in extra:
                        ev = mybir.InstEventSemaphore(
                            name=f"waitsplit-{ctr}", ins=[], outs=[]
                        )
                        ctr += 1
                        ev.engine = inst.engine
                        ev.sync_info = mybir.SyncInfo(on_wait=[w], on_update=[])
                        evs.append(ev)
                    si.on_wait = keep
                    il[i:i] = evs
                    i += len(evs)
                i += 1
    return ctr


def _ternary(w):
    w = np.asarray(w, np.float32)
    return np.where(np.abs(w) < THRESH, 0.0, np.sign(w)).astype(np.float32)


def _pack_weight(w):
    """[out_f, in_f] f32 -> ternarized, transposed, tiled fp16
    [jc, p, ko, j] with element = tern(w)[jc*128+j, ko*128+p]."""
    of, inf_ = w.shape
    jc, ko = of // P, inf_ // P
    t = _ternary(w).reshape(jc, P, ko, P)  # [jc, j, ko, p]
    t = np.ascontiguousarray(t.transpose(0, 3, 2, 1))  # [jc, p, ko, j]
    return t.astype(np.float16)


def _split_f16(a):
    """f32 array -> (hi, lo) fp16 pair with hi + lo ~= a (22-bit mantissa)."""
    hi = a.astype(np.float16)
    lo = (a - hi.astype(np.float32)).astype(np.float16)
    return hi, lo


def _pack_x(x_shard):
    """[B, DIM] f32 -> [2, p, ko, b] fp16 (hi, lo)."""
    b, inf_ = x_shard.shape
    xt = np.ascontiguousarray(
        x_shard.reshape(b, inf_ // P, P).transpose(2, 1, 0)
    ).astype(np.float32)  # [p, ko, b]
    hi, lo = _split_f16(xt)
    return np.ascontiguousarray(np.stack([hi, lo], axis=0))


def _pack_bias_col(b):
    """[out_f] -> [128, out_f//128] (partition-major)."""
    return np.ascontiguousarray(np.asarray(b, np.float32).reshape(-1, P).T)


def _build_nc():
    nc = bass.Bass()

    xT = nc.declare_dram_parameter("xT", [2, P, JC_D, B], F16, isOutput=False)
    wf = nc.declare_dram_parameter("wf", [JC_D, P, JC_D, P], F16, isOutput=False)
    wc = nc.declare_dram_parameter("wc", [JC_D, P, JC_D, P], F16, isOutput=False)
    wg = nc.declare_dram_parameter("wg", [JC_D, P, JC_D, P], F16, isOutput=False)
    wo = nc.declare_dram_parameter("wo", [JC_D, P, JC_D, P], F16, isOutput=False)
    wu = nc.declare_dram_parameter("wu", [JC_H, P, JC_D, P], F16, isOutput=False)
    wg2 = nc.declare_dram_parameter("wg2", [JC_H, P, JC_D, P], F16, isOutput=False)
    wo2 = nc.declare_dram_parameter("wo2", [JC_D, 2, P, JC_H // 2, P], F16, isOutput=False)
    biases = nc.declare_dram_parameter("biases", [P, N_BIAS_COLS], F32, isOutput=False)
    out = nc.declare_dram_parameter("out", [JC_D, P, B], F32, isOutput=True)

    AF = mybir.ActivationFunctionType
    from contextlib import ExitStack

    with tile.TileContext(nc) as tc:
        with (
            tc.tile_pool(name="const", bufs=1) as const,
            tc.tile_pool(name="wpool", bufs=4) as wpool,
            tc.tile_pool(name="psum", bufs=6, space="PSUM") as psum,
        ):
            bias_sb = const.tile([P, N_BIAS_COLS], F32)
            nc.sync.dma_start(out=bias_sb[:], in_=biases[:])

            def bias_ap(col):
                return bias_sb[:, col : col + 1]

            def mm_split(ps, w_sb, rhs_sb, nk):
                """Accumulate over nk chunks x 2 (hi+lo) splits.
                rhs_sb is [P, 2, nk, B]; same weight tile reused for hi/lo."""
                for ko in range(nk):
                    for s in range(2):
                        nc.tensor.matmul(
                            ps,
                            w_sb[:, ko],
                            rhs_sb[:, s, ko],
                            start=(ko == 0 and s == 0),
                            stop=(ko == nk - 1 and s == 1),
                        )

            es_gh = ExitStack()
            gh_pool = es_gh.enter_context(tc.tile_pool(name="gh_pool", bufs=1))
            gh_sb = gh_pool.tile([P, 2, JC_D, B], F16)
            # o lives on the right-side stack so its (phase 2..3) lifetime can
            # straddle the left-side gh (1..2) and gu (3..4) lifetimes with
            # every pool release still LIFO per side.
            es_o = ExitStack()

            # ---- phase 1: MLGRU gates; gh = g * ((1-f)*c) -> hi/lo fp16 ----
            with (
                tc.tile_pool(name="x_pool", bufs=1) as x_pool,
                tc.tile_pool(name="tmp1", bufs=2) as tmp,
            ):
                x_sb = x_pool.tile([P, 2, JC_D, B], F16)
                nc.sync.dma_start(out=x_sb[:, 0], in_=xT[0])
                nc.sync.dma_start(out=x_sb[:, 1], in_=xT[1])

                for jc in range(JC_D):
                    wf_sb = wpool.tile([P, JC_D, P], F16, tag="w512")
                    nc.sync.dma_start(out=wf_sb[:], in_=wf[jc])
                    ps_f = psum.tile([P, B], F32, tag="ps")
                    mm_split(ps_f, wf_sb, x_sb, JC_D)

                    wc_sb = wpool.tile([P, JC_D, P], F16, tag="w512")
                    nc.sync.dma_start(out=wc_sb[:], in_=wc[jc])
                    ps_c = psum.tile([P, B], F32, tag="ps")
                    mm_split(ps_c, wc_sb, x_sb, JC_D)

                    wg_sb = wpool.tile([P, JC_D, P], F16, tag="w512")
                    nc.sync.dma_start(out=wg_sb[:], in_=wg[jc])
                    ps_g = psum.tile([P, B], F32, tag="ps")
                    mm_split(ps_g, wg_sb, x_sb, JC_D)

                    # 1-f = sigmoid(-(t_f + b_f)); bias column holds -b_f
                    onemf = tmp.tile([P, B], F32, tag="onemf")
                    nc.scalar.activation(
                        onemf, ps_f, AF.Sigmoid, bias=bias_ap(COL_NF + jc), scale=-1.0
                    )
                    c_sb = tmp.tile([P, B], F32, tag="c")
                    nc.scalar.activation(c_sb, ps_c, AF.Silu, bias=bias_ap(COL_C + jc))
                    g_sb = tmp.tile([P, B], F32, tag="g")
                    nc.scalar.activation(g_sb, ps_g, AF.Sigmoid, bias=bias_ap(COL_G + jc))
                    h_sb = tmp.tile([P, B], F32, tag="h")
                    nc.vector.tensor_mul(h_sb, onemf, c_sb)
                    ghf = tmp.tile([P, B], F32, tag="ghf")
                    nc.vector.tensor_mul(ghf, g_sb, h_sb)
                    nc.vector.tensor_copy(gh_sb[:, 0, jc], ghf)
                    ghr = tmp.tile([P, B], F32, tag="ghr")
                    nc.vector.tensor_sub(ghr, ghf, gh_sb[:, 0, jc])
                    nc.vector.tensor_copy(gh_sb[:, 1, jc], ghr)

            o_pool = es_o.enter_context(
                tc.tile_pool(name="o_pool", bufs=1, side="right")
            )
            o_sb = o_pool.tile([P, 2, JC_D, B], F16)

            # ---- phase 2: o = out_proj(gh) + b -> hi/lo fp16 ----
            with tc.tile_pool(name="tmp2", bufs=2) as tmp:
                for jc in range(JC_D):
                    wo_sb = wpool.tile([P, JC_D, P], F16, tag="w512")
                    nc.sync.dma_start(out=wo_sb[:], in_=wo[jc])
                    ps_o = psum.tile([P, B], F32, tag="ps")
                    mm_split(ps_o, wo_sb, gh_sb, JC_D)
                    of = tmp.tile([P, B], F32, tag="of")
                    nc.vector.tensor_scalar_add(of, ps_o, bias_ap(COL_O + jc))
                    nc.vector.tensor_copy(o_sb[:, 0, jc], of)
                    orr = tmp.tile([P, B], F32, tag="orr")
                    nc.vector.tensor_sub(orr, of, o_sb[:, 0, jc])
                    nc.vector.tensor_copy(o_sb[:, 1, jc], orr)
            es_gh.close()

            es_gu = ExitStack()
            gu_pool = es_gu.enter_context(tc.tile_pool(name="gu_pool", bufs=1))
            gu_sb = gu_pool.tile([P, JC_H, B], F16)

            # ---- phase 3: BitGLU gu = sigmoid(.)*silu(.) / 16 -> fp16 ----
            with tc.tile_pool(name="tmp3", bufs=2) as tmp:
                for hc in range(JC_H):
                    wu_sb = wpool.tile([P, JC_D, P], F16, tag="w512")
                    nc.sync.dma_start(out=wu_sb[:], in_=wu[hc])
                    ps_u = psum.tile([P, B], F32, tag="ps")
                    mm_split(ps_u, wu_sb, o_sb, JC_D)

                    wg2_sb = wpool.tile([P, JC_D, P], F16, tag="w512")
                    nc.sync.dma_start(out=wg2_sb[:], in_=wg2[hc])
                    ps_g2 = psum.tile([P, B], F32, tag="ps")
                    mm_split(ps_g2, wg2_sb, o_sb, JC_D)

                    u_sb = tmp.tile([P, B], F32, tag="u")
                    nc.scalar.activation(u_sb, ps_u, AF.Silu, bias=bias_ap(COL_U + hc))
                    g2_sb = tmp.tile([P, B], F32, tag="g2")
                    nc.scalar.activation(g2_sb, ps_g2, AF.Sigmoid, bias=bias_ap(COL_G2 + hc))
                    guf = tmp.tile([P, B], F32, tag="guf")
                    nc.vector.tensor_mul(guf, g2_sb, u_sb)
                    nc.vector.tensor_scalar_mul(gu_sb[:, hc], guf, 1.0 / GU_SCALE)
            es_o.close()

            # ---- phase 4: y = proj_out(gu)*16 + b ----
            with (
                tc.tile_pool(name="wpool2", bufs=2) as wpool2,
                tc.tile_pool(name="outp", bufs=2) as outp,
            ):
                for jc in range(JC_D):
                    ps_y = psum.tile([P, B], F32, tag="ps")
                    for half in range(2):
                        wo2_sb = wpool2.tile([P, JC_H // 2, P], F16, tag="w2m")
                        nc.sync.dma_start(out=wo2_sb[:], in_=wo2[jc, half])
                        for kk in range(JC_H // 2):
                            hc = half * (JC_H // 2) + kk
                            nc.tensor.matmul(
                                ps_y,
                                wo2_sb[:, kk],
                                gu_sb[:, hc],
                                start=(hc == 0),
                                stop=(hc == JC_H - 1),
                            )
                    y_sb = outp.tile([P, B], F32, tag="y")
                    nc.vector.tensor_scalar(
                        y_sb, ps_y, GU_SCALE, bias_ap(COL_Y + jc),
                        mybir.AluOpType.mult, mybir.AluOpType.add,
                    )
                    nc.sync.dma_start(out=out[jc], in_=y_sb[:])
            es_gu.close()

    _split_excess_waits(nc)
    return nc


def prep_in_maps(inputs):
    """Build the 8 per-core input maps from the full-size inputs."""
    x = np.asarray(inputs["x"], np.float32)

    wo2_packed = _pack_weight(inputs["proj_out_w"])  # [JC_D, P, JC_H, P]
    wo2_packed = np.ascontiguousarray(
        wo2_packed.reshape(JC_D, P, 2, JC_H // 2, P).transpose(0, 2, 1, 3, 4)
    )  # [JC_D, 2, P, JC_H//2, P]

    shared = {
        "wf": _pack_weight(inputs["f_gate_w"]),
        "wc": _pack_weight(inputs["c_proj_w"]),
        "wg": _pack_weight(inputs["g_gate_w"]),
        "wo": _pack_weight(inputs["out_proj_w"]),
        "wu": _pack_weight(inputs["proj_u_w"]),
        "wg2": _pack_weight(inputs["proj_g_w"]),
        "wo2": wo2_packed,
    }
    bias = np.zeros((P, N_BIAS_COLS), np.float32)
    bias[:, COL_NF:COL_NF + JC_D] = _pack_bias_col(-np.asarray(inputs["f_gate_b"]))
    bias[:, COL_C:COL_C + JC_D] = _pack_bias_col(inputs["c_proj_b"])
    bias[:, COL_G:COL_G + JC_D] = _pack_bias_col(inputs["g_gate_b"])
    bias[:, COL_O:COL_O + JC_D] = _pack_bias_col(inputs["out_proj_b"])
    bias[:, COL_U:COL_U + JC_H] = _pack_bias_col(inputs["proj_u_b"])
    bias[:, COL_G2:COL_G2 + JC_H] = _pack_bias_col(inputs["proj_g_b"])
    bias[:, COL_Y:COL_Y + JC_D] = _pack_bias_col(inputs["proj_out_b"])
    shared["biases"] = bias

    in_maps = []
    for core in range(NCORES):
        m = dict(shared)
        m["xT"] = _pack_x(x[core * B : (core + 1) * B])
        in_maps.append(m)
    return in_maps


def gather_output(results):
    """results[i]['out'] is [JC_D, P, B]; assemble full [BATCH, DIM] f32."""
    parts = []
    for core in range(NCORES):
        y = np.asarray(results[core]["out"], np.float32)  # [jc, p, b]
        parts.append(y.reshape(DIM, B).T)  # [b, j]
    return np.ascontiguousarray(np.concatenate(parts, axis=0))


def run(inputs, trace=False, **kw):
    from concourse.bass_utils import run_bass_kernel_spmd

    nc = _build_nc()
    in_maps = prep_in_maps(inputs)
    res = run_bass_kernel_spmd(nc, in_maps, core_ids=list(range(NCORES)), trace=trace, **kw)
    return res


def kernel(**inputs):
    res = run(inputs, trace=False)
    return gather_output(res.results)


# revision 12
# speedup vs baseline: 1.6609x; 1.6609x over previous
"""Trainium2 Bass kernel for nn_BitModel (MLGRU step + BitGLU, ternary weights).

Strategy: pure data-parallel over the 4096 batch dim (512 rows per core,
zero collectives). Weights are ternarized exactly in f32 on the host
(sign(w) * (|w| >= 0.33)); the ternary values {-1, 0, +1} are exactly
representable in fp16, so device matmuls run at full 1-cycle/row TensorE
rate with zero weight quantization error.

Precision scheme (validated against the reference in numpy emulation,
rel err ~2e-4): activations that feed matmuls are split into fp16 hi+lo
pairs (x, gh, o), which doubles those contraction loops but keeps ~22-bit
effective mantissa; gu is a single fp16 stream pre-scaled by 1/16 (range),
un-scaled in the output epilogue. All accumulation is f32 in PSUM.

On-device dataflow is feature-major: every tensor lives in SBUF as
[128 partitions = feature % 128, feature_chunk, batch=512]. Each BitLinear is
out[j, b] = sum_k Wt[k, j] * act[k, b]; lhsT = pre-transposed weight tile
[k_part, j] (stationary), rhs = activation tile [k_part, batch] (moving,
N=512 = one PSUM bank). No transposes anywhere on device.
"""

import sys

sys.path.insert(0, "/opt/trn_rl_repo")

import numpy as np

import concourse.bass as bass
import concourse.mybir as mybir
import concourse.tile as tile
from concourse.vector_clock import ScopedClock

DIM = 2048
HID = 8192
BATCH = 4096
NCORES = 8
B = BATCH // NCORES  # 512 batch rows per core
P = 128
JC_D = DIM // P  # 16 feature chunks for DIM
JC_H = HID // P  # 64 feature chunks for HID
THRESH = 0.33
GU_SCALE = 16.0  # gu is stored as gu/16 in fp16 to stay inside fp16 range

F16 = mybir.dt.float16
F32 = mybir.dt.float32

# bias column layout in the packed [128, 208] bias tensor
COL_NF = 0  # -f_gate_b (negated: we compute 1-f = sigmoid(-(t+b)))
COL_C = 16
COL_G = 32
COL_O = 48
COL_U = 64
COL_G2 = 128
COL_Y = 192
N_BIAS_COLS = 208


def _patch_tile_drain():
    """This walrus build rejects instructions carrying >~2 attached sem
    waits ("Too many sync wait commands") and Tile's kernel-tail drain
    carries one wait per active logical proc. Re-emit those waits as
    standalone wait_ge instructions (1 wait each) before a wait-free
    drain."""
    if getattr(tile.TileContext, "_drain_patched", False):
        return

    def _drain_and_barrier(self, tick_clock, wait_clock):
        nc = self.nc
        probe = nc.sync.nop(nofuse=True)
        wait_clock.add_sem_waits(
            probe.ins, ScopedClock({None: tick_clock.global_clock})
        )
        si = probe.ins.sync_info
        waits = list(si.on_wait) if si else []
        if si:
            si.on_wait = []
        handles = {h.name: h for h in self.sems.allocated().values()}
        for w in waits:
            nc.sync.wait_ge(handles[w.ant_name], w.wait_value)
        nc.sync.drain()
        nc.all_engine_barrier()
        assert self.sems is not None
        popped = nc._tile_sem_poison_stack.pop()
        assert popped is self._sem_poison
        nc.clear_and_free_semaphores(list(self.sems.allocated().values()))
        nc.all_engine_barrier()

    tile.TileContext._drain_and_barrier = _drain_and_barrier
    tile.TileContext._drain_patched = True


_patch_tile_drain()


def _split_excess_waits(nc, cap=1):
    """This walrus build rejects instructions carrying more than ~2 attached
    sem waits ("Too many sync wait commands"). Tile attaches one wait per
    depended-on logical proc. Rewrite every instruction with >cap waits into
    a chain of single-wait InstEventSemaphore ops (what raw wait_ge emits,
    known-good) followed by the instruction keeping only `cap` waits."""
    ctr = 0
    for f in nc.m.functions:
        for bb in f.blocks:
            il = bb.instructions
            i = 0
            while i < len(il):
                inst = il[i]
                si = inst.sync_info
                waits = list(si.on_wait) if si else []
                if len(waits) > cap:
                    extra, keep = waits[:-cap], waits[-cap:]
                    evs = []
                    for w in extra:
                        ev = mybir.InstEventSemaphore(
                            name=f"waitsplit-{ctr}", ins=[], outs=[]
                        )
                        ctr += 1
                        ev.engine = inst.engine
                        ev.sync_info = mybir.SyncInfo(on_wait=[w], on_update=[])
                        evs.append(ev)
                    si.on_wait = keep
                    il[i:i] = evs
                    i += len(evs)
                i += 1
    return ctr


def _ternary(w):
    w = np.asarray(w, np.float32)
    return np.where(np.abs(w) < THRESH, 0.0, np.sign(w)).astype(np.float32)


def _pack_weight(w):
    """[out_f, in_f] f32 -> ternarized, transposed, tiled fp16
    [jc, p, ko, j] with element = tern(w)[jc*128+j, ko*128+p]."""
    of, inf_ = w.shape
    jc, ko = of // P, inf_ // P
    t = _ternary(w).reshape(jc, P, ko, P)  # [jc, j, ko, p]
    t = np.ascontiguousarray(t.transpose(0, 3, 2, 1))  # [jc, p, ko, j]
    return t.astype(np.float16)


def _split_f16(a):
    """f32 array -> (hi, lo) fp16 pair with hi + lo ~= a (22-bit mantissa)."""
    hi = a.astype(np.float16)
    lo = (a - hi.astype(np.float32)).astype(np.float16)
    return hi, lo


def _pack_x(x_shard):
    """[B, DIM] f32 -> [2, p, ko, b] fp16 (hi, lo)."""
    b, inf_ = x_shard.shape
    xt = np.ascontiguousarray(
        x_shard.reshape(b, inf_ // P, P).transpose(2, 1, 0)
    ).astype(np.float32)  # [p, ko, b]
    hi, lo = _split_f16(xt)
    return np.ascontiguousarray(np.stack([hi, lo], axis=0))


def _pack_bias_col(b):
    """[out_f] -> [128, out_f//128] (partition-major)."""
    return np.ascontiguousarray(np.asarray(b, np.float32).reshape(-1, P).T)


def _build_nc():
    nc = bass.Bass()

    xT = nc.declare_dram_parameter("xT", [2, P, JC_D, B], F16, isOutput=False)
    wf = nc.declare_dram_parameter("wf", [JC_D, P, JC_D, P], F16, isOutput=False)
    wc = nc.declare_dram_parameter("wc", [JC_D, P, JC_D, P], F16, isOutput=False)
    wg = nc.declare_dram_parameter("wg", [JC_D, P, JC_D, P], F16, isOutput=False)
    wo = nc.declare_dram_parameter("wo", [JC_D, P, JC_D, P], F16, isOutput=False)
    wu = nc.declare_dram_parameter("wu", [JC_H, P, JC_D, P], F16, isOutput=False)
    wg2 = nc.declare_dram_parameter("wg2", [JC_H, P, JC_D, P], F16, isOutput=False)
    wo2 = nc.declare_dram_parameter("wo2", [JC_D, 2, P, JC_H // 2, P], F16, isOutput=False)
    biases = nc.declare_dram_parameter("biases", [P, N_BIAS_COLS], F32, isOutput=False)
    out = nc.declare_dram_parameter("out", [JC_D, P, B], F32, isOutput=True)

    AF = mybir.ActivationFunctionType
    from contextlib import ExitStack

    with tile.TileContext(nc) as tc:
        with (
            tc.tile_pool(name="const", bufs=1) as const,
            tc.tile_pool(name="wpool", bufs=4) as wpool,
            tc.tile_pool(name="psum", bufs=6, space="PSUM") as psum,
        ):
            bias_sb = const.tile([P, N_BIAS_COLS], F32)
            nc.sync.dma_start(out=bias_sb[:], in_=biases[:])

            def bias_ap(col):
                return bias_sb[:, col : col + 1]

            def mm_split(ps, w_sb, rhs_sb, nk):
                """Accumulate over nk chunks x 2 (hi+lo) splits.
                rhs_sb is [P, 2, nk, B]; same weight tile reused for hi/lo."""
                for ko in range(nk):
                    for s in range(2):
                        nc.tensor.matmul(
                            ps,
                            w_sb[:, ko],
                            rhs_sb[:, s, ko],
                            start=(ko == 0 and s == 0),
                            stop=(ko == nk - 1 and s == 1),
                        )

            es_gh = ExitStack()
            gh_pool = es_gh.enter_context(tc.tile_pool(name="gh_pool", bufs=1))
            gh_sb = gh_pool.tile([P, 2, JC_D, B], F16)
            # o lives on the right-side stack so its (phase 2..3) lifetime can
            # straddle the left-side gh (1..2) and gu (3..4) lifetimes with
            # every pool release still LIFO per side.
            es_o = ExitStack()

            # ---- phase 1: MLGRU gates; gh = g * ((1-f)*c) -> hi/lo fp16 ----
            with (
                tc.tile_pool(name="x_pool", bufs=1) as x_pool,
                tc.tile_pool(name="tmp1", bufs=2) as tmp,
            ):
                x_sb = x_pool.tile([P, 2, JC_D, B], F16)
                nc.sync.dma_start(out=x_sb[:, 0], in_=xT[0])
                nc.sync.dma_start(out=x_sb[:, 1], in_=xT[1])

                for jc in range(JC_D):
                    wf_sb = wpool.tile([P, JC_D, P], F16, tag="w512")
                    nc.sync.dma_start(out=wf_sb[:], in_=wf[jc])
                    ps_f = psum.tile([P, B], F32, tag="ps")
                    mm_split(ps_f, wf_sb, x_sb, JC_D)

                    wc_sb = wpool.tile([P, JC_D, P], F16, tag="w512")
                    nc.sync.dma_start(out=wc_sb[:], in_=wc[jc])
                    ps_c = psum.tile([P, B], F32, tag="ps")
                    mm_split(ps_c, wc_sb, x_sb, JC_D)

                    wg_sb = wpool.tile([P, JC_D, P], F16, tag="w512")
                    nc.sync.dma_start(out=wg_sb[:], in_=wg[jc])
                    ps_g = psum.tile([P, B], F32, tag="ps")
                    mm_split(ps_g, wg_sb, x_sb, JC_D)

                    # 1-f = sigmoid(-(t_f + b_f)); bias column holds -b_f
                    onemf = tmp.tile([P, B], F32, tag="onemf")
                    nc.scalar.activation(
                        onemf, ps_f, AF.Sigmoid, bias=bias_ap(COL_NF + jc), scale=-1.0
                    )
                    c_sb = tmp.tile([P, B], F32, tag="c")
                    nc.scalar.activation(c_sb, ps_c, AF.Silu, bias=bias_ap(COL_C + jc))
                    g_sb = tmp.tile([P, B], F32, tag="g")
                    nc.scalar.activation(g_sb, ps_g, AF.Sigmoid, bias=bias_ap(COL_G + jc))
                    h_sb = tmp.tile([P, B], F32, tag="h")
                    nc.vector.tensor_mul(h_sb, onemf, c_sb)
                    ghf = tmp.tile([P, B], F32, tag="ghf")
                    nc.vector.tensor_mul(ghf, g_sb, h_sb)
                    nc.vector.tensor_copy(gh_sb[:, 0, jc], ghf)
                    ghr = tmp.tile([P, B], F32, tag="ghr")
                    nc.vector.tensor_sub(ghr, ghf, gh_sb[:, 0, jc])
                    nc.vector.tensor_copy(gh_sb[:, 1, jc], ghr)

            o_pool = es_o.enter_context(
                tc.tile_pool(name="o_pool", bufs=1, side="right")
            )
            o_sb = o_pool.tile([P, JC_D, B], F16)

            # ---- phase 2: o = out_proj(gh) + b -> single fp16 ----
            with tc.tile_pool(name="tmp2", bufs=2) as tmp:
                for jc in range(JC_D):
                    wo_sb = wpool.tile([P, JC_D, P], F16, tag="w512")
                    nc.sync.dma_start(out=wo_sb[:], in_=wo[jc])
                    ps_o = psum.tile([P, B], F32, tag="ps")
                    mm_split(ps_o, wo_sb, gh_sb, JC_D)
                    nc.vector.tensor_scalar_add(o_sb[:, jc], ps_o, bias_ap(COL_O + jc))
            es_gh.close()

            es_gu = ExitStack()
            gu_pool = es_gu.enter_context(tc.tile_pool(name="gu_pool", bufs=1))
            gu_sb = gu_pool.tile([P, JC_H, B], F16)

            # ---- phase 3: BitGLU gu = sigmoid(.)*silu(.) / 16 -> fp16 ----
            with tc.tile_pool(name="tmp3", bufs=2) as tmp:
                for hc in range(JC_H):
                    wu_sb = wpool.tile([P, JC_D, P], F16, tag="w512")
                    nc.sync.dma_start(out=wu_sb[:], in_=wu[hc])
                    ps_u = psum.tile([P, B], F32, tag="ps")
                    for ko in range(JC_D):
                        nc.tensor.matmul(
                            ps_u, wu_sb[:, ko], o_sb[:, ko],
                            start=(ko == 0), stop=(ko == JC_D - 1),
                        )

                    wg2_sb = wpool.tile([P, JC_D, P], F16, tag="w512")
                    nc.sync.dma_start(out=wg2_sb[:], in_=wg2[hc])
                    ps_g2 = psum.tile([P, B], F32, tag="ps")
                    for ko in range(JC_D):
                        nc.tensor.matmul(
                            ps_g2, wg2_sb[:, ko], o_sb[:, ko],
                            start=(ko == 0), stop=(ko == JC_D - 1),
                        )

                    u_sb = tmp.tile([P, B], F32, tag="u")
                    nc.scalar.activation(u_sb, ps_u, AF.Silu, bias=bias_ap(COL_U + hc))
                    g2_sb = tmp.tile([P, B], F32, tag="g2")
                    nc.scalar.activation(g2_sb, ps_g2, AF.Sigmoid, bias=bias_ap(COL_G2 + hc))
                    guf = tmp.tile([P, B], F32, tag="guf")
                    nc.vector.tensor_mul(guf, g2_sb, u_sb)
                    nc.vector.tensor_scalar_mul(gu_sb[:, hc], guf, 1.0 / GU_SCALE)
            es_o.close()

            # ---- phase 4: y = proj_out(gu)*16 + b ----
            with (
                tc.tile_pool(name="wpool2", bufs=2) as wpool2,
                tc.tile_pool(name="outp", bufs=2) as outp,
            ):
                for jc in range(JC_D):
                    ps_y = psum.tile([P, B], F32, tag="ps")
                    for half in range(2):
                        wo2_sb = wpool2.tile([P, JC_H // 2, P], F16, tag="w2m")
                        nc.sync.dma_start(out=wo2_sb[:], in_=wo2[jc, half])
                        for kk in range(JC_H // 2):
                            hc = half * (JC_H // 2) + kk
                            nc.tensor.matmul(
                                ps_y,
                                wo2_sb[:, kk],
                                gu_sb[:, hc],
                                start=(hc == 0),
                                stop=(hc == JC_H - 1),
                            )
                    y_sb = outp.tile([P, B], F32, tag="y")
                    nc.vector.tensor_scalar(
                        y_sb, ps_y, GU_SCALE, bias_ap(COL_Y + jc),
                        mybir.AluOpType.mult, mybir.AluOpType.add,
                    )
                    nc.sync.dma_start(out=out[jc], in_=y_sb[:])
            es_gu.close()

    _split_excess_waits(nc)
    return nc


def prep_in_maps(inputs):
    """Build the 8 per-core input maps from the full-size inputs."""
    x = np.asarray(inputs["x"], np.float32)

    wo2_packed = _pack_weight(inputs["proj_out_w"])  # [JC_D, P, JC_H, P]
    wo2_packed = np.ascontiguousarray(
        wo2_packed.reshape(JC_D, P, 2, JC_H // 2, P).transpose(0, 2, 1, 3, 4)
    )  # [JC_D, 2, P, JC_H//2, P]

    shared = {
        "wf": _pack_weight(inputs["f_gate_w"]),
        "wc": _pack_weight(inputs["c_proj_w"]),
        "wg": _pack_weight(inputs["g_gate_w"]),
        "wo": _pack_weight(inputs["out_proj_w"]),
        "wu": _pack_weight(inputs["proj_u_w"]),
        "wg2": _pack_weight(inputs["proj_g_w"]),
        "wo2": wo2_packed,
    }
    bias = np.zeros((P, N_BIAS_COLS), np.float32)
    bias[:, COL_NF:COL_NF + JC_D] = _pack_bias_col(-np.asarray(inputs["f_gate_b"]))
    bias[:, COL_C:COL_C + JC_D] = _pack_bias_col(inputs["c_proj_b"])
    bias[:, COL_G:COL_G + JC_D] = _pack_bias_col(inputs["g_gate_b"])
    bias[:, COL_O:COL_O + JC_D] = _pack_bias_col(inputs["out_proj_b"])
    bias[:, COL_U:COL_U + JC_H] = _pack_bias_col(inputs["proj_u_b"])
    bias[:, COL_G2:COL_G2 + JC_H] = _pack_bias_col(inputs["proj_g_b"])
    bias[:, COL_Y:COL_Y + JC_D] = _pack_bias_col(inputs["proj_out_b"])
    shared["biases"] = bias

    in_maps = []
    for core in range(NCORES):
        m = dict(shared)
        m["xT"] = _pack_x(x[core * B : (core + 1) * B])
        in_maps.append(m)
    return in_maps


def gather_output(results):
    """results[i]['out'] is [JC_D, P, B]; assemble full [BATCH, DIM] f32."""
    parts = []
    for core in range(NCORES):
        y = np.asarray(results[core]["out"], np.float32)  # [jc, p, b]
        parts.append(y.reshape(DIM, B).T)  # [b, j]
    return np.ascontiguousarray(np.concatenate(parts, axis=0))


def run(inputs, trace=False, **kw):
    from concourse.bass_utils import run_bass_kernel_spmd

    nc = _build_nc()
    in_maps = prep_in_maps(inputs)
    res = run_bass_kernel_spmd(nc, in_maps, core_ids=list(range(NCORES)), trace=trace, **kw)
    return res


def kernel(**inputs):
    res = run(inputs, trace=False)
    return gather_output(res.results)
